# revision 35
# baseline (speedup 1.0000x reference)
"""Trainium2 Bass kernel: single-head attention module (dense transformer).

Computes, for x [4, 4096, 256] (f32) and per-projection weights/biases:
    q = x @ Wq + bq;  k = x @ Wk + bk;  v = x @ Wv + bv
    out = softmax((q k^T) / sqrt(256)) @ v @ Wo + bo
Sharding over 8 NeuronCores: core c handles batch c//2, query half c%2
(the host rotates each core's batch so its queries are rows 0..2047).

Algebraic restructure (weight-weight folds are 256^3 host flops, exact):
  - scores = x_q (Wq Wk^T) x_k^T + per-key bias d; M = Wq Wk^T removes the
    K projection; x^T (bf16) is the key operand.
  - out = (P x_k) (Wv Wo) / denom + (bv Wo + bo): removes the V projection;
    Wvo = Wv Wo folds the two output projections into one.

Precision split (measured rel err ~1.5e-2 vs the fp32 reference, tol 2e-2;
budget derived in err_sim.py against the reference data):
  - scores stay bf16: e4m3 on either score operand alone costs 1.1-2.7e-2
    of final error, while fp8 DoubleRow only doubles PE throughput on this
    hardware (157 TF/s, 1.0 cycles/row at K=256 - the public cost model's
    0.5 c/r is optimistic 2x), so any compensated split cancels the gain.
  - PV runs fully in fp8 e4m3 via DoubleRow matmuls: the exp writes P
    directly as e4m3 into key-tile PAIR tiles [128, 2x512], natural-layout
    x ships host-prequantized to e4m3, and each (pair, e-chunk) is one
    K=256 DoubleRow. The softmax denominator is computed from the same
    e4m3 P values so the quantization's weighted-mean component cancels.
    The exp bias carries a -1.0 softmax-invariant shift keeping P inside
    e4m3 range (max 240).
  - two of every 16 pairs skip ACT's exp: DVE computes them with a
    Schraudolph-style one-op approx exp (affine then saturating
    round-to-nearest uint8 convert = the e4m3 bitpattern; bow-corrected
    by SCHR_SHIFT). This keeps the ACT engine's exp stream (the #2
    engine) well below the PE so its latency never back-pressures.
  - PE work per core: G 8k cycles, scores^T 131k (bf16), PV 66k (fp8 DR),
    denominator ~10k, projection/transposes ~10k -> ~225k cycles, ~94 us
    busy at 2.4 GHz.

Scheduling (hand-interleaved in emission order = per-engine exec order):
  - PE warmup matmuls over disjoint PSUM slices fill the initial DMA wait
    (the tensor engine p-state ramp reaches full clock only on
    back-to-back pre-satisfied work; extra mid-stream filler matmuls
    measurably HURT real hardware and are avoided).
  - input DMAs are paced to first consumers: 512-column pieces of the x^T
    query half feed each G matmul, natural-x (fp8) pieces feed the PV
    stream, the x^T key half streams in behind the early score slots.
  - scores/exp run ahead of PV: the exps for key tiles 2p,2p+1 fill pair
    tile p at pair-slot p; the PV DoubleRow pair runs at pair-slot p+2
    (the previous block's acc evictions land at pair-slot 0, so the
    acc-bank WAR clears just ahead of the first PV pair).
  - denominator: each fp8 pair is summed elementwise to bf16, then two
    interleaved bf16 chains accumulate pairs 0..12 (even pairs on DVE,
    odd on Pool, which cannot touch PSUM); at the drain the PE closes the
    denominator with two bf16 ones-matmuls over the chains plus three
    fp8 ones-pair DoubleRow pieces (pairs 13..15), so no chain op ever
    gates the PE just-in-time.
  - the denominator row is evicted and transposed on the PE (four
    [1,128]->[128,1] transpose pieces in ONE PSUM accumulation group -
    separate start=True writes would re-arm the bank's pending-zero
    region and wipe earlier columns) into per-partition layout; its DVE
    reciprocal recT then scales each projected output tile by 1/denom
    inside the eviction itself: DVE scalar_tensor_tensor computes
    proj*recT + bo in one op. The tail (last block) moves half of this
    to the otherwise-idle ACT engine: a plain-f32 rank-1 denom x bo
    matmul folds bo into PSUM so ACT's eviction is a Copy scaled by recT.
  - each block's denominator fetch/transpose/reciprocal and its four
    projection+eviction quarters interleave into the NEXT block's
    pair-slots 0..5; acc PSUM banks free early via plain unscaled
    copies to SBUF (one bank per e-chunk suffices, keeping PSUM at
    3 score + 2 acc + 1 accd + 2 shared G/proj banks = 8). Outputs
    evict as bf16 (the host upcasts) halving output DMA bytes.
"""

import numpy as np

import concourse.bass as bass  # noqa: F401
import concourse.tile as tile
from concourse import bacc, mybir
from concourse.bass_utils import run_bass_kernel_spmd
from concourse.masks import make_identity  # noqa: F401

B, S, D = 4, 4096, 256
SQ = S // 2  # queries per core
NCORES = 8
F32 = mybir.dt.float32
F32R = mybir.dt.float32r
BF16 = mybir.dt.bfloat16
FP8 = mybir.dt.float8e4
SCALE = 1.0 / 16.0  # 1/sqrt(D)
PSHIFT = 1.0  # softmax-invariant exp shift: P = exp(s - PSHIFT) stays in e4m3
SCHR_SHIFT = -0.06  # Schraudolph bow-correction shift (tuned in err_sim.py)
SCHR_A = (1.0 / 16.0) * 8.0 / np.log(2.0)  # byte = sp*SCHR_A + dschr
EXP = mybir.ActivationFunctionType.Exp
COPY = mybir.ActivationFunctionType.Copy
DR = mybir.MatmulPerfMode.DoubleRow


def _r(ap):
    """View an fp32 AP as float32r: full-rate fp32 matmul on the PE."""
    return ap.bitcast(F32R)


def _build():
    nc = bacc.Bacc("TRN2", target_bir_lowering=False, debug=False,
                   num_devices=NCORES)

    # natural-layout x arrives pre-quantized to e4m3 (PV stationary
    # operand); x^T arrives bf16 (scores operand). Both are pure host-side
    # data marshaling, like the per-core batch rotation.
    xkv = nc.dram_tensor("xkv", [S, D], FP8, kind="ExternalInput").ap()
    xkvt_dram = nc.dram_tensor("xkvt", [D, S], BF16,
                               kind="ExternalInput").ap()
    m_dram = nc.dram_tensor("mqk", [D, D], BF16, kind="ExternalInput").ap()
    wvo_dram = nc.dram_tensor("wvo", [D, D], F32, kind="ExternalInput").ap()
    dpos_dram = nc.dram_tensor("dpos", [128, 32], F32,
                               kind="ExternalInput").ap()
    dschr_dram = nc.dram_tensor("dschr", [128, 32], F32,
                                kind="ExternalInput").ap()
    bo_dram = nc.dram_tensor("bo", [D], F32, kind="ExternalInput").ap()
    out = nc.dram_tensor("out", [SQ, D], BF16,
                         kind="ExternalOutput").ap()

    bo_row = bo_dram.rearrange("(a b) -> a b", a=1)  # [1, 256]
    xkv_g = xkv.rearrange("(g j p) c -> g p j c", j=8, p=128)   # [4,128,8,256]
    xkvt_c = xkvt_dram.rearrange("(c p) k -> c p k", p=128)     # [2,128,4096]
    m_g = m_dram.rearrange("(j p) c -> p j c", j=2)
    wvo_g = wvo_dram.rearrange("(j p) c -> p j c", j=2)
    out_t = out.rearrange("(t p) c -> t p c", p=128)            # [16,128,256]

    with tile.TileContext(nc) as tc:
        with (
            tc.tile_pool(name="const", bufs=1) as cpool,
            tc.tile_pool(name="pt", bufs=5) as pt_pool,
            tc.tile_pool(name="sacc", bufs=8) as sacc_pool,
            tc.tile_pool(name="ovec", bufs=2) as ovec_pool,
            tc.tile_pool(name="rct", bufs=2) as rct_pool,
            tc.tile_pool(name="fout", bufs=2) as fout_pool,
            tc.tile_pool(name="psc", bufs=1, space="PSUM") as psc,
            tc.tile_pool(name="psacc", bufs=1, space="PSUM") as psacc,
            tc.tile_pool(name="psx", bufs=1, space="PSUM") as psx,
        ):
            # ---- constants (no DMA deps) ----
            warm = cpool.tile([128, 128], F32R, tag="warm", name="warm")
            nc.gpsimd.memset(warm[:].bitcast(mybir.dt.uint32), 0x3F800000)
            ones128 = cpool.tile([128, 128], BF16, tag="ones128",
                                 name="ones128")
            # fp8 all-ones pair tile: lhsT of the denominator DoubleRow
            # pieces (e4m3 1.0 = 0x38)
            ones8 = cpool.tile([128, 2, 128], FP8, tag="ones8", name="ones8")
            ones1 = cpool.tile([1, 128], F32, tag="ones1", name="ones1")
            ident1 = cpool.tile([1, 1], F32, tag="id1", name="id1")

            # ---- PE warmup: dummy matmuls during the initial DMA window so
            # the tensor engine p-state ramp completes before real work.
            # Writes rotate over disjoint PSUM slices. ----
            wps = psacc.tile([128, 512], F32, tag="accd", name="accd",
                             bufs=1)
            wi = [0]

            def warmup(n):
                for _ in range(n):
                    s = (wi[0] % 4) * 128
                    nc.tensor.matmul(wps[:, s:s + 128], warm[:], warm[:],
                                     start=True, stop=True)
                    wi[0] += 1

            warmup(13)

            # ---- input tiles + DMA order (earliest consumer first) ----
            xt = [cpool.tile([128, 8 * D], FP8, tag=f"xin{g}", name=f"xin{g}")
                  for g in range(4)]
            m_sb = cpool.tile([128, 2 * D], BF16, tag="m", name="m")
            wvo_sb = cpool.tile([128, 2 * D], F32R, tag="wvo", name="wvo")
            dpos = cpool.tile([128, 32], F32, tag="dpos", name="dpos")
            dschr = cpool.tile([128, 32], F32, tag="dschr", name="dschr")
            bo_sb = cpool.tile([1, D], F32, tag="bor", name="bor")

            xkvT = [cpool.tile([128, S], BF16, tag=f"xkvT{c}", name=f"xkvT{c}")
                    for c in range(2)]
            G = [cpool.tile([128, SQ], BF16, tag=f"G{c}", name=f"G{c}")
                 for c in range(2)]

            # DMA order, paced to first consumers: 512-column pieces of the
            # x^T query half feed each G matmul; natural-x fp8 pieces feed
            # the PV stream; the x^T key half streams behind early slots.
            # two HWDGE queues: SP streams the scores operands (M, x^T)
            # with nothing else in the way; ACT's queue carries the
            # PV-side natural-x, biases and Wvo in parallel.
            xt0j = xt[0].rearrange("p (j c) -> p j c", j=8)
            nc.sync.dma_start(
                m_sb.rearrange("p (j c) -> p j c", j=2), m_g[:])
            for c in range(2):
                nc.sync.dma_start(xkvT[c][:, 0:512], xkvt_c[c][:, 0:512])
            nc.sync.dma_start(dpos[:], dpos_dram)
            nc.sync.dma_start(dschr[:], dschr_dram)
            nc.sync.dma_start(bo_sb[:], bo_row[:])
            for c in range(2):
                nc.sync.dma_start(xkvT[c][:, 512:1024],
                                  xkvt_c[c][:, 512:1024])
            nc.sync.dma_start(xt0j[:, 0:4], xkv_g[0][:, 0:4])
            for c in range(2):
                nc.sync.dma_start(xkvT[c][:, 1024:2048],
                                  xkvt_c[c][:, 1024:2048])
            nc.sync.dma_start(xt0j[:, 4:8], xkv_g[0][:, 4:8])
            for c in range(2):
                nc.sync.dma_start(xkvT[c][:, 2048:4096],
                                  xkvt_c[c][:, 2048:4096])
            for g in (1, 2, 3):
                nc.sync.dma_start(
                    xt[g].rearrange("p (j c) -> p j c", j=8), xkv_g[g])
            nc.sync.dma_start(
                wvo_sb.rearrange("p (j c) -> p j c", j=2), _r(wvo_g[:]))
            # bo broadcast across partitions: the in1 of every output
            # eviction's scalar_tensor_tensor (out = proj*rec + bo)
            bob = cpool.tile([128, D], F32, tag="bob", name="bob")

            def bo_bcast():
                bps = psx.tile([128, 512], F32, tag="px", name="px", bufs=2)
                nc.tensor.matmul(bps[:, 0:D], ones1[:], bo_sb[:],
                                 start=True, stop=True)
                nc.vector.tensor_copy(bob[:], bps[:, 0:D])

            def qmt_grp(blk, c2):
                # G[c2][:, 512-query block] = (M^T x_q^T) e-chunk c2
                qsl = slice(blk * 512, (blk + 1) * 512)
                pp = psx.tile([128, 512], F32, tag="px", name="px", bufs=2)
                for j in range(2):
                    nc.tensor.matmul(
                        pp[:],
                        m_sb[:, j * D + c2 * 128: j * D + (c2 + 1) * 128],
                        xkvT[j][:, qsl],
                        start=(j == 0), stop=(j == 1))
                nc.vector.tensor_copy(G[c2][:, qsl], pp[:])

            def fp_t4(ctx, t4, on_act=False):
                # projection of one 128-query tile from the UNSCALED o
                # tiles; 1/denom is applied per-partition by the eviction.
                # DVE path: out = proj*recT + bo via one STT. ACT path
                # (tail only, DVE is the tail bottleneck): bo enters the
                # PSUM as a rank-1 denom x bo matmul so that the scaled
                # Copy eviction yields (proj + denom*bo)*rec = proj*rec+bo.
                tsl = slice(t4 * 128, (t4 + 1) * 128)
                fpt = psx.tile([128, 512], F32, tag="px", name="px", bufs=2)
                fp = fpt[:, 0:D]
                if on_act:
                    # f32 rank-1 (4 c/r but tiny; tail PE is idle)
                    nc.tensor.matmul(fp, ctx["drow"][0:1, tsl], bo_sb[:],
                                     start=True, stop=False)
                for e in range(2):
                    nc.tensor.matmul(
                        fp, ctx["o"][e][:, tsl],
                        wvo_sb[:, e * D:(e + 1) * D],
                        start=(not on_act and e == 0), stop=(e == 1))
                fo = fout_pool.tile([128, D], BF16, tag="fout",
                                    name="fout", bufs=4)
                if on_act:
                    nc.scalar.activation(fo[:], fp, COPY,
                                         scale=ctx["recT"][:, t4:t4 + 1])
                    nc.scalar.dma_start(out_t[ctx["qoff"] // 128 + t4],
                                        fo[:])
                else:
                    nc.vector.scalar_tensor_tensor(
                        fo[:], fp, ctx["recT"][:, t4:t4 + 1], bob[:],
                        mybir.AluOpType.mult, mybir.AluOpType.add)
                    nc.sync.dma_start(out_t[ctx["qoff"] // 128 + t4], fo[:])

            def denom_fetch(ctx, on_act=False):
                # accd row -> SBUF; the transpose to per-partition layout
                # happens on the PE (denom_transpose). The tail uses ACT
                # (idle there) so the PE's transposes unblock sooner.
                ctx["drow"] = rct_pool.tile([1, 512], F32, tag="drow",
                                            name="drow", bufs=2)
                if on_act:
                    nc.scalar.copy(ctx["drow"][:], ctx["accd"][0:1, :])
                else:
                    nc.vector.tensor_copy(ctx["drow"][:],
                                          ctx["accd"][0:1, :])

            def denom_transpose(ctx):
                # four [1,128]->[128,1] PE transposes into one PSUM
                # accumulation group (start only on the first: later
                # pieces must not re-arm the bank's pending-zero region)
                tp = psx.tile([128, 512], F32, tag="px", name="px", bufs=2)
                ctx["recTsrc"] = tp
                for t4 in range(4):
                    nc.tensor.matmul(
                        tp[:, t4:t4 + 1],
                        ctx["drow"][0:1, t4 * 128:(t4 + 1) * 128],
                        ident1[:], is_transpose=True,
                        start=(t4 == 0), stop=(t4 == 3),
                        skip_group_check=True)

            def o_fetch(ctx, on_act=False):
                # unscaled acc -> SBUF (frees acc's PSUM bank for the next
                # block's PV accumulation); the tail uses ACT (idle there)
                ctx["o"] = [ovec_pool.tile([128, 512], F32R, tag=f"o{e}",
                                            name=f"o{e}") for e in range(2)]
                for e in range(2):
                    if on_act:
                        nc.scalar.copy(ctx["o"][e][:],
                                       ctx["acc"][e][:, 0:512])
                    else:
                        nc.vector.tensor_copy(ctx["o"][e][:],
                                              ctx["acc"][e][:, 0:512])

            def recT_compute(ctx):
                ctx["recT"] = rct_pool.tile([128, 4], F32, tag="rt",
                                            name="rt", bufs=2)
                nc.vector.reciprocal(ctx["recT"][:],
                                     ctx["recTsrc"][:, 0:4])

            # ---- prologue: only G block 0 gates the first score slot ----
            qmt_grp(0, 0)
            qmt_grp(0, 1)
            # non-critical const memsets AFTER the prologue so DVE's queue
            # reaches the G evictions (which gate the first scores) sooner
            nc.vector.memset(ones128[:].bitcast(mybir.dt.uint16), 0x3F80)
            nc.vector.memset(ones8[:].bitcast(mybir.dt.uint8), 0x38)
            nc.vector.memset(ones1[:], 1.0)
            nc.vector.memset(ident1[:], 1.0)

            extras = {}

            def add_extra(qb, ps, th):
                extras.setdefault((qb, ps), []).append(th)

            add_extra(0, 3, bo_bcast)
            pslot = 4
            for blk in (1, 2, 3):
                for c2 in range(2):
                    add_extra(0, pslot,
                              lambda blk=blk, c2=c2: qmt_grp(blk, c2))
                    pslot += 1

            # pairs 13,15 -> DVE schraudolph (Pool cannot read PSUM; the
            # split placement keeps DVE's ps13-15 window under one op per
            # pair-slot so the drain's pv15/DR15 never wait on it)
            PAIR_ENG = {13: nc.vector, 15: nc.vector}

            blocks = [(0, 512), (512, 512), (1024, 512), (1536, 512)]
            ctxs = []
            for bi, (qoff, w) in enumerate(blocks):
                qsl = slice(qoff, qoff + w)
                acc = [psacc.tile([128, 512], F32, tag=f"acc{e}",
                                  name=f"acc{e}", bufs=1) for e in range(2)]
                accd = psacc.tile([128, 512], F32, tag="accd", name="accd",
                                  bufs=1)
                ctx = {"qoff": qoff, "w": w, "acc": acc, "accd": accd}
                ctxs.append(ctx)
                prev = ctxs[bi - 1] if bi >= 1 else None

                pts = {}
                chains = {0: None, 1: None}

                def chain_pair(p, w=w, chains=chains, pts=pts):
                    # denominator: fp8 pair-sum to bf16, then two
                    # interleaved bf16 chains (even pairs on DVE, odd on
                    # Pool, which cannot touch PSUM). Pairs 13..15 skip the
                    # chain: they join as DoubleRow pieces on the PE.
                    par = p % 2
                    eng = nc.vector if par == 0 else nc.gpsimd
                    ps_t = sacc_pool.tile([128, 512], BF16, tag="sacc",
                                          name="sacc", bufs=8)
                    eng.tensor_add(ps_t[:, 0:w], pts[p][:, 0:512][:, 0:w],
                                   pts[p][:, 512:1024][:, 0:w])
                    if chains[par] is None:
                        chains[par] = ps_t
                    else:
                        t = sacc_pool.tile([128, 512], BF16, tag="sacc",
                                           name="sacc", bufs=8)
                        eng.tensor_add(t[:, 0:w], chains[par][:, 0:w],
                                       ps_t[:, 0:w])
                        chains[par] = t

                def pv_pair(p, acc=acc, w=w, pts=pts):
                    # fp8 DoubleRow over a key-tile pair: K=256 keys per
                    # instruction, one per e-chunk
                    g, j0 = (2 * p) // 8, (2 * p) % 8
                    stat2 = xt[g][:, j0 * D:(j0 + 2) * D].rearrange(
                        "q (t c) -> q t c", t=2)
                    rhs2 = pts[p][:, 0:1024].rearrange("q (t c) -> q t c",
                                                       t=2)
                    for e in range(2):
                        nc.tensor.matmul(
                            acc[e][:, 0:w],
                            stat2[:, :, e * 128:(e + 1) * 128],
                            rhs2[:, :, 0:w],
                            start=(p == 0), stop=(p == 15),
                            perf_mode=DR)

                def boundary(ps):
                    # previous block's denominator fetch/scale/projection
                    if ps == 0:
                        denom_fetch(prev)
                        o_fetch(prev)
                    elif ps == 1:
                        denom_transpose(prev)
                        recT_compute(prev)
                    elif ps in (2, 3, 4, 5):
                        fp_t4(prev, ps - 2)

                # scores/exp run ahead of PV + denominator so the PE never
                # waits on the activation engine's exp latency. One wide
                # [128, 1024] exp serves a key-tile pair.
                for ps in range(16):
                    for th in extras.get((bi, ps), ()):
                        th()
                    if 2 <= ps <= 14:
                        chain_pair(ps - 2)
                    pt = pt_pool.tile([128, 1024], FP8, tag="pt", name="pt",
                                      bufs=5)
                    pts[ps] = pt
                    for half in range(2):
                        st = 2 * ps + half
                        ssl = slice(st * 128, (st + 1) * 128)
                        sp = psc.tile([128, 512], F32, tag="sc", name="sc",
                                      bufs=3)
                        nc.tensor.matmul(sp[:, 0:w], xkvT[0][:, ssl],
                                         G[0][:, qsl], start=True,
                                         stop=False)
                        nc.tensor.matmul(sp[:, 0:w], xkvT[1][:, ssl],
                                         G[1][:, qsl], start=False,
                                         stop=True)
                        ph = pt[:, half * 512:half * 512 + w]
                        eng = PAIR_ENG.get(ps)
                        if eng is None:
                            nc.scalar.activation(ph, sp[:, 0:w], EXP,
                                                 scale=SCALE,
                                                 bias=dpos[:, st:st + 1])
                        else:
                            # Schraudolph exp-to-e4m3: one affine +
                            # saturating round-to-nearest uint8 convert
                            # builds the e4m3 bitpattern directly (rel err
                            # ~3% on these pairs; uniform shift tuned in
                            # err_sim.py). Offloads ACT's exp stream.
                            eng.tensor_scalar(
                                ph.bitcast(mybir.dt.uint8), sp[:, 0:w],
                                float(SCHR_A), dschr[:, st:st + 1],
                                mybir.AluOpType.mult, mybir.AluOpType.add)
                    if ps >= 2:
                        pv_pair(ps - 2)
                    if prev is not None:
                        boundary(ps)
                # drain: the last three PV pairs, then close the
                # denominator on the PE: two bf16 ones-matmuls over the
                # chains plus three fp8 ones-pair DoubleRow pieces.
                # merge the two chains on DVE (emitted after the last
                # schraudolph so it never delays pv15's input), then one
                # bf16 ones-matmul placed LAST in the drain covers it
                sm = sacc_pool.tile([128, 512], BF16, tag="sacc",
                                    name="sacc", bufs=8)
                nc.vector.tensor_add(sm[:, 0:w], chains[0][:, 0:w],
                                     chains[1][:, 0:w])
                pv_pair(14)
                rhs13 = pts[13][:, 0:1024].rearrange("q (t c) -> q t c", t=2)
                nc.tensor.matmul(accd[:, 0:w], ones8[:], rhs13[:, :, 0:w],
                                 start=True, stop=False, perf_mode=DR)
                pv_pair(15)
                for p in (14, 15):
                    rhs2 = pts[p][:, 0:1024].rearrange("q (t c) -> q t c",
                                                       t=2)
                    nc.tensor.matmul(accd[:, 0:w], ones8[:],
                                     rhs2[:, :, 0:w], start=False,
                                     stop=False, perf_mode=DR)
                nc.tensor.matmul(accd[:, 0:w], ones128[:],
                                 sm[:, 0:w], start=False, stop=True)

            # ---- epilogue: last block's fetch/scale/projection ----
            last = ctxs[-1]
            denom_fetch(last)
            denom_transpose(last)
            o_fetch(last, on_act=True)
            recT_compute(last)
            for t4 in range(4):
                fp_t4(last, t4, on_act=(t4 % 2 == 0))

    nc.compile()
    return nc


_NC = None


def _get_nc():
    global _NC
    if _NC is None:
        _NC = _build()
    return _NC


def _make_in_maps(x, Wq, bq, Wk, bk, Wv, bv, Wo, bo):
    """Host-side prep: weight folds + per-core data marshaling.

    M = Wq Wk^T and Wvo = Wv Wo are exact weight-weight folds; bv folds
    into bo (attention rows sum to 1); the only non-softmax-invariant bias
    is the per-key d = x_k (Wk bq), shipped pre-tiled/pre-scaled in dpos
    together with the -PSHIFT e4m3-range shift. x ships in natural layout
    pre-quantized to e4m3 (PV stationary operand) and pre-transposed in
    bf16 (scores operand) - pure layout/dtype marshaling."""
    import ml_dtypes
    bf16 = ml_dtypes.bfloat16
    e4m3 = ml_dtypes.float8_e4m3
    M = (Wq @ Wk.T).astype(bf16)
    Wvo = (Wv @ Wo).astype(np.float32)
    bo_eff = (bv @ Wo + bo).astype(np.float32)
    u = (Wk @ bq).astype(np.float32)
    in_maps = []
    for c in range(NCORES):
        b, h = divmod(c, 2)
        xb = x[b] if h == 0 else np.ascontiguousarray(
            np.concatenate([x[b, SQ:], x[b, :SQ]]))
        d = (xb @ u) * np.float32(SCALE) - np.float32(PSHIFT)
        dpos = np.ascontiguousarray(d.reshape(32, 128).T).astype(np.float32)
        dschr = ((dpos + np.float32(SCHR_SHIFT)) * np.float32(8.0 / np.log(2.0))
                 + np.float32(56.0)).astype(np.float32)
        in_maps.append({
            "xkv": xb.astype(e4m3),
            "xkvt": np.ascontiguousarray(xb.astype(bf16).T),
            "mqk": M, "wvo": Wvo, "dpos": dpos, "dschr": dschr,
            "bo": bo_eff,
        })
    return in_maps


class _Runner:
    """Cached jitted SPMD executor (run_bass_kernel_spmd rebuilds its jax
    closure every call, forcing a retrace; this traces once)."""

    def __init__(self, nc):
        import jax
        from jax.sharding import Mesh, PartitionSpec
        from jax.experimental.shard_map import shard_map
        from concourse import bass2jax, mybir as mb

        bass2jax.install_neuronx_cc_hook()
        self.jax = jax
        if not any("axon" in str(getattr(d, "platform", "")).lower()
                   or str(d).startswith("NC_")
                   for d in jax.devices()):
            import jax._src.xla_bridge as xb
            jax.config.update("jax_platforms", None)
            xb._clear_backends()
            if hasattr(xb.get_backend, "cache_clear"):
                xb.get_backend.cache_clear()
            if not any("axon" in str(getattr(d, "platform", "")).lower()
                       or str(d).startswith("NC_")
                       for d in jax.devices()):
                jax.config.update("jax_platforms", "axon")
                xb._clear_backends()
                if hasattr(xb.get_backend, "cache_clear"):
                    xb.get_backend.cache_clear()
        partition_name = (nc.partition_id_tensor.name
                          if nc.partition_id_tensor else None)
        in_names, out_names, out_avals = [], [], []
        for alloc in nc.m.functions[0].allocations:
            if not isinstance(alloc, mb.MemoryLocationSet):
                continue
            name = alloc.memorylocations[0].name
            if alloc.kind == "ExternalInput":
                if name != partition_name:
                    in_names.append(name)
            elif alloc.kind == "ExternalOutput":
                out_names.append(name)
                out_avals.append(jax.core.ShapedArray(
                    tuple(alloc.tensor_shape), mb.dt.np(alloc.dtype)))
        self.in_names, self.out_names, self.out_avals = \
            in_names, out_names, out_avals
        n_params, n_outs = len(in_names), len(out_names)
        bind_in_names = in_names + out_names + (
            [partition_name] if partition_name else [])

        def _body(*args):
            operands = list(args)
            if partition_name is not None:
                operands.append(bass2jax.partition_id_tensor())
            outs = bass2jax._bass_exec_p.bind(
                *operands,
                out_avals=tuple(out_avals),
                in_names=tuple(bind_in_names),
                out_names=tuple(out_names),
                lowering_input_output_aliases=(),
                sim_require_finite=True,
                sim_require_nnan=True,
                nc=nc,
            )
            return tuple(outs)

        devices = jax.devices()[:NCORES]
        mesh = Mesh(np.asarray(devices), ("core",))
        spec = (PartitionSpec("core"),) * (n_params + n_outs)
        self.fn = jax.jit(
            shard_map(_body, mesh=mesh, in_specs=spec,
                      out_specs=(PartitionSpec("core"),) * n_outs,
                      check_rep=False),
            donate_argnums=tuple(range(n_params, n_params + n_outs)),
            keep_unused=True,
        )

    def run(self, in_maps):
        concat_in = [
            np.concatenate([np.asarray(m[n]) for m in in_maps], axis=0)
            for n in self.in_names
        ]
        concat_zeros = [
            np.zeros((NCORES * a.shape[0], *a.shape[1:]), a.dtype)
            for a in self.out_avals
        ]
        outs = self.fn(*concat_in, *concat_zeros)
        return [
            {n: np.asarray(outs[i]).reshape(NCORES, *self.out_avals[i].shape)[c]
             for i, n in enumerate(self.out_names)}
            for c in range(NCORES)
        ]


_RUNNER = None


def _get_runner():
    global _RUNNER
    if _RUNNER is None:
        _RUNNER = _Runner(_get_nc())
    return _RUNNER


def kernel(**inputs):
    x = np.ascontiguousarray(np.asarray(inputs["x"], dtype=np.float32))
    Wq = np.ascontiguousarray(np.asarray(inputs["Wq"], dtype=np.float32))
    Wk = np.ascontiguousarray(np.asarray(inputs["Wk"], dtype=np.float32))
    Wv = np.ascontiguousarray(np.asarray(inputs["Wv"], dtype=np.float32))
    Wo = np.ascontiguousarray(np.asarray(inputs["Wo"], dtype=np.float32))
    bq = np.ascontiguousarray(np.asarray(inputs["bq"], dtype=np.float32))
    bk = np.ascontiguousarray(np.asarray(inputs["bk"], dtype=np.float32))
    bv = np.ascontiguousarray(np.asarray(inputs["bv"], dtype=np.float32))
    bo = np.ascontiguousarray(np.asarray(inputs["bo"], dtype=np.float32))

    try:
        runner = _get_runner()
    except Exception:
        runner = None
    in_maps = _make_in_maps(x, Wq, bq, Wk, bk, Wv, bv, Wo, bo)
    results = None
    if runner is not None:
        try:
            results = runner.run(in_maps)
        except Exception:
            results = None
    if results is None:
        results = run_bass_kernel_spmd(
            _get_nc(), in_maps, core_ids=list(range(NCORES))).results
    outp = np.empty((B, S, D), dtype=np.float32)
    for c in range(NCORES):
        b, h = divmod(c, 2)
        outp[b, h * SQ:(h + 1) * SQ] = results[c]["out"].astype(np.float32)
    return outp
